# revision 58
# baseline (speedup 1.0000x reference)
"""BiLSTM-CRF loss kernel for Trainium2 (8 NeuronCores, data-parallel over batch).

v2 design (per core, BL=16 sequences):
  Phase 1 (512 wall-steps): fwd LSTM (t=s) and bwd LSTM (t=511-s) run as two
    interleaved streams so every engine pipelines across streams.
    - Gate order host-permuted to [i,f,o,g] so one Sigmoid op covers i,f,o.
    - Input projection xw is windowed (32 steps) and injected into the gate
      PSUM accumulation via an identity matmul (PE is cheap, DVE is not).
    - Emissions pre-biased once; both streams add their w_out matmul via a
      single joint DVE op per step.
    - Only Sigmoid/Tanh on ACT -> zero activation-table thrash.
    - Gold-score (unary+transition) reductions interleaved into phase-1 slack.
  Phase 2 (~256 wall-steps): CRF log-partition via two interleaved exp-space
    recursions on bulk-exp'd emissions: alpha upward (t=1..256) and beta
    downward (t=510..256), meeting at t*=256. Periodic rescale; the log
    compensation terms are stored and Ln'd once at the end.
"""

import numpy as np

PAD_IDX = 0
VOCAB, K, E, H = 30000, 20, 256, 256
B, T = 128, 512
NCORES = 8
BL = B // NCORES          # 16 sequences per core
WIN = 32                  # proj window (time steps)
NW = T // WIN             # 16 windows
RESC = 8                  # CRF rescale interval (wall steps)
TME = 256                 # CRF meeting point: logZ = ln(sum D_TME * B_TME)
NRESC = 31                # rescales at k=8,16,...,248

_cache = {}


def _build_program():
    from contextlib import ExitStack
    import concourse.bass as bass
    import concourse.bacc as bacc
    import concourse.tile as tile
    from concourse import mybir
    from concourse.masks import make_identity

    f32 = mybir.dt.float32
    bf16 = mybir.dt.bfloat16
    i32 = mybir.dt.int32
    u8 = mybir.dt.uint8
    AF = mybir.ActivationFunctionType
    OP = mybir.AluOpType

    nc = bacc.Bacc(None, target_bir_lowering=False, debug=False)
    names = {}

    with ExitStack() as ctx:
        tc = ctx.enter_context(tile.TileContext(nc))
        dram = ctx.enter_context(tc.tile_pool(name="dram", bufs=1, space="DRAM"))

        def din(key, shape, dt=f32):
            t = dram.tile(shape, dt, kind="ExternalInput", name=key)
            names[key] = t.tensor.name
            return t

        emb = din("emb", [VOCAB, E], bf16)
        toks = din("toks", [T * BL, 1], i32)            # window-major token ids
        masku = din("masku", [1, T * BL], u8)           # col = t*16+b
        invmu = din("invmu", [1, T * BL], u8)           # 1 - mask
        t1h = din("t1h", [K, T * BL], bf16)             # one-hot(tag)*mask
        tnx = din("tnx", [K, T * BL], bf16)             # shifted one-hot*mask
        lnmask = din("lnmask", [1, (NRESC + 1) * 2 * BL])  # mask at rescale times
        wih = {d: din(f"wih_{d}", [E, 4 * H], bf16) for d in "fb"}
        whh = {d: din(f"whh_{d}", [E, 4 * H], bf16) for d in "fb"}
        bih = {d: din(f"bih_{d}", [128, 8]) for d in "fb"}
        bihT = {d: din(f"bihT_{d}", [1, 8 * 128], bf16) for d in "fb"}
        woutT = din("woutT", [4, 128, K], bf16)         # chunks: Fk0,Fk1,Bk0,Bk1
        bout = din("bout", [K, 1])
        expA = din("expA", [K, K], bf16)                # exp(transition)
        expAT = din("expAT", [K, K], bf16)              # exp(transition).T
        Abf = din("Abf", [K, K], bf16)                  # transition (bf16)
        wstar = din("wstar", [1, K])                    # 1^T (expA^T)^{-1}
        out_loss = dram.tile([1, BL], f32, kind="ExternalOutput")
        names["out"] = out_loss.tensor.name

        sg = ctx.enter_context(tc.tile_pool(name="sg", bufs=1))       # singles
        tmp = ctx.enter_context(tc.tile_pool(name="tmp", bufs=4))     # step temps
        gat = ctx.enter_context(tc.tile_pool(name="gat", bufs=4))     # gather tiles
        fin = ctx.enter_context(tc.tile_pool(name="fin", bufs=3))     # finalize
        ps_g = ctx.enter_context(tc.tile_pool(name="ps_g", bufs=2, space="PSUM"))
        ps_w = ctx.enter_context(tc.tile_pool(name="ps_w", bufs=1, space="PSUM"))
        ps_t = ctx.enter_context(tc.tile_pool(name="ps_t", bufs=1, space="PSUM"))
        ps_s = ctx.enter_context(tc.tile_pool(name="ps_s", bufs=2, space="PSUM"))

        # ---- resident SBUF tensors ----
        s_wih = {d: sg.tile([128, 2, 4 * H], bf16, tag=f"wih{d}", name=f"wih{d}") for d in "fb"}
        s_whh = {d: sg.tile([128, 2, 4 * H], bf16, tag=f"whh{d}", name=f"whh{d}") for d in "fb"}
        for d in "fb":
            nc.sync.dma_start(out=s_wih[d][:], in_=wih[d][:].rearrange("(k p) m -> p k m", p=128))
            nc.sync.dma_start(out=s_whh[d][:], in_=whh[d][:].rearrange("(k p) m -> p k m", p=128))
        s_bih = {d: sg.tile([128, 8], f32, tag=f"bih{d}", name=f"bih{d}") for d in "fb"}
        s_bihT = {d: sg.tile([1, 8, 128], bf16, tag=f"bihT{d}", name=f"bihT{d}") for d in "fb"}
        for d in "fb":
            nc.sync.dma_start(out=s_bih[d][:], in_=bih[d][:])
            nc.sync.dma_start(out=s_bihT[d][:].rearrange("o m p -> o (m p)"), in_=bihT[d][:])
        onesb = sg.tile([1, 512], bf16, tag="onesb")
        nc.vector.memset(onesb[:], 1.0)
        s_wout = sg.tile([128, 4, K], bf16, tag="wout")
        nc.sync.dma_start(out=s_wout[:], in_=woutT[:].rearrange("c p k -> p c k"))
        s_bout = sg.tile([K, 1], f32, tag="bout")
        nc.sync.dma_start(out=s_bout[:], in_=bout[:])
        s_eA = sg.tile([K, K], bf16, tag="eA")
        nc.sync.dma_start(out=s_eA[:], in_=expA[:])
        s_eAT = sg.tile([K, K], bf16, tag="eAT")
        nc.sync.dma_start(out=s_eAT[:], in_=expAT[:])
        s_A = sg.tile([K, K], bf16, tag="A")
        nc.sync.dma_start(out=s_A[:], in_=Abf[:])
        s_ws = sg.tile([1, K], f32, tag="ws")
        nc.sync.dma_start(out=s_ws[:], in_=wstar[:])
        s_t1h = sg.tile([K, T * BL], bf16, tag="t1h")
        nc.sync.dma_start(out=s_t1h[:], in_=t1h[:])
        s_tnx = sg.tile([K, T * BL], bf16, tag="tnx")
        nc.sync.dma_start(out=s_tnx[:], in_=tnx[:])
        s_lnm = sg.tile([1, (NRESC + 1) * 2 * BL], f32, tag="lnm")
        nc.sync.dma_start(out=s_lnm[:], in_=lnmask[:])

        ones = sg.tile([128, K], f32, tag="ones")
        nc.vector.memset(ones[:], 1.0)
        ident = sg.tile([128, 128], bf16, tag="ident")
        make_identity(nc, ident[:])

        # mask replica: (128, T, BL), col = t*16+b, broadcast across partitions
        maskrep = sg.tile([128, T, BL], u8, tag="maskrep")
        nc.sync.dma_start(
            out=maskrep[:],
            in_=bass.AP(tensor=masku.tensor, offset=masku[:].offset,
                        ap=[[0, 128], [BL, T], [1, BL]]),
        )
        invrep = sg.tile([128, T, BL], u8, tag="invrep")
        nc.sync.dma_start(
            out=invrep[:],
            in_=bass.AP(tensor=invmu.tensor, offset=invmu[:].offset,
                        ap=[[0, 128], [BL, T], [1, BL]]),
        )
        neg50 = sg.tile([128, 1], bf16, tag="neg50")
        nc.vector.memset(neg50[:], -50.0)

        # token indices for gathers (128 per column)
        NT128 = T * BL // 128
        idxall = sg.tile([128, NT128], i32, tag="idxall")
        nc.sync.dma_start(out=idxall[:],
                          in_=bass.AP(tensor=toks.tensor, offset=toks[:].offset,
                                      ap=[[1, 128], [128, NT128]]))

        # emissions (f32) and their exp; pre-bias emit with b_out
        emit = sg.tile([K, T, BL], f32, tag="emit")
        emf0 = emit[:].rearrange("k t b -> k (t b)")
        nc.vector.memset(emf0, 0.0)
        nc.vector.tensor_scalar_add(emf0, emf0, s_bout[:, 0:1])
        expE = emit  # exp taken in place after phase 1 (unary reads are done by then)

        # per-direction rotating transposed-input windows and projection windows
        xtp = ctx.enter_context(tc.tile_pool(name="xtp", bufs=2))
        winp = ctx.enter_context(tc.tile_pool(name="winp", bufs=2))

        # LSTM states, layout [128, dir(2), k(2), BL]
        st_h = sg.tile([128, 2, 2, BL], bf16, tag="st_h")
        st_c = sg.tile([128, 2, 2, BL], f32, tag="st_c")
        nc.vector.memset(st_h[:], 0.0)
        nc.vector.memset(st_c[:], 0.0)

        # CRF state S[:, 0]=alpha D, S[:, 1]=beta B; ln-compensation buffer
        S = sg.tile([K, 2, BL], f32, tag="S")
        lnbuf = sg.tile([1, NRESC + 1, 2, BL], f32, tag="lnbuf")
        Uacc = sg.tile([K, BL], f32, tag="Uacc")
        nc.vector.memset(Uacc[:], 0.0)
        TRacc = sg.tile([K, BL], f32, tag="TRacc")
        nc.vector.memset(TRacc[:], 0.0)

        def gather_piece(xT, w, g, d):
            """Gather+transpose one 128-token piece of window w."""
            j = w * 4 + g
            xg = gat.tile([128, E], bf16, tag="xg")
            nc.gpsimd.indirect_dma_start(
                out=xg[:], out_offset=None, in_=emb[:],
                in_offset=bass.IndirectOffsetOnAxis(ap=idxall[:, j:j + 1], axis=0),
            )
            for k in range(2):
                pst = ps_t.tile([128, 128], bf16, tag="pst")
                nc.tensor.transpose(out=pst[:], in_=xg[:, k * 128:(k + 1) * 128],
                                    identity=ident[:])
                if (g + k) % 2 == 0:
                    nc.vector.tensor_copy(xT[:, k, g * 128:(g + 1) * 128], pst[:])
                else:
                    nc.scalar.activation(xT[:, k, g * 128:(g + 1) * 128], pst[:], AF.Copy)

        def proj_chunk(win, xT, d, m):
            """One m-chunk of the window projection (+bias via matmul)."""
            psw = ps_w.tile([128, 512], f32, tag="psw")
            for k in range(2):
                nc.tensor.matmul(psw[:], lhsT=s_wih[d][:, k, m * 128:(m + 1) * 128],
                                 rhs=xT[:, k, :], start=(k == 0), stop=False)
            nc.tensor.matmul(psw[:], lhsT=s_bihT[d][:, m, :], rhs=onesb[:],
                             start=False, stop=True)
            wv = win[:, m].rearrange("p t b -> p (t b)")
            if m % 2 == 0:
                nc.scalar.activation(wv, psw[:], AF.Copy)
            else:
                nc.vector.tensor_copy(wv, psw[:])

        def clamp_window(win, w):
            """Clamp i-gate pre-activation to -50 at padded positions (bwd)."""
            pred = bass.AP(tensor=invrep.tensor,
                           offset=invrep[0:128, w * WIN, 0:BL].offset,
                           ap=[invrep[:].ap[0], [0, 2], [BL, WIN], [1, BL]])
            data = bass.AP(tensor=neg50.tensor, offset=neg50[:].offset,
                           ap=[neg50[:].ap[0], [0, 2], [0, WIN], [0, BL]])
            nc.vector.copy_predicated(win[:, 0:2], pred, data)

        def full_window(w, d):
            xT = xtp.tile([128, 2, 512], bf16, tag=f"xT{d}", name=f"xT{d}")
            for g in range(4):
                gather_piece(xT, w, g, d)
            win = winp.tile([128, 8, WIN, BL], bf16, tag=f"win{d}", name=f"win{d}")
            for m in range(8):
                proj_chunk(win, xT, d, m)
            if d == "b":
                clamp_window(win, w)
            return win

        # mask AP helpers --------------------------------------------------
        def mask_joint(tlo, thi, parts, reps):
            """(parts, 2, reps, BL) AP over maskrep: dir0 at t=tlo, dir1 at t=thi."""
            base = maskrep[0:parts, tlo, 0:BL]
            return bass.AP(tensor=base.tensor, offset=base.offset,
                           ap=[base.ap[0], [(thi - tlo) * BL, 2], [0, reps], [1, BL]])

        def mask_one(t, parts, reps):
            base = maskrep[0:parts, t, 0:BL]
            if reps == 1:
                return base
            return bass.AP(tensor=base.tensor, offset=base.offset,
                           ap=[base.ap[0], [0, reps], [1, BL]])

        # -------- gold-score chunk work (interleaved into phase 1) --------
        NCH = 16
        CW = T * BL // NCH          # 512 cols per chunk

        def trans_chunk(ci):
            """TRacc += reduce_t(A[tag, :] * shifted-onehot) for chunk ci."""
            psa = ps_w.tile([K, CW], f32, tag="psw", name="psa")
            nc.tensor.matmul(psa[:], lhsT=s_A[:], rhs=s_t1h[:, ci * CW:(ci + 1) * CW],
                             start=True, stop=True)
            um = fin.tile([K, CW], f32, tag="um")
            nc.vector.tensor_tensor(um[:], psa[:], s_tnx[:, ci * CW:(ci + 1) * CW], op=OP.mult)
            ur = fin.tile([K, BL], f32, tag="ur")
            umr = bass.AP(tensor=um.tensor, offset=um[:].offset,
                          ap=[um[:].ap[0], [1, BL], [BL, CW // BL]])
            nc.vector.tensor_reduce(ur[:], umr, axis=mybir.AxisListType.X, op=OP.add)
            nc.vector.tensor_tensor(TRacc[:], TRacc[:], ur[:], op=OP.add)

        def unary_chunk(ci):
            """Uacc += reduce_t(onehot * emit) for chunk ci (emit must be complete)."""
            um = fin.tile([K, CW], f32, tag="um")
            ef = emit[:].rearrange("k t b -> k (t b)")
            nc.gpsimd.tensor_tensor(um[:], s_t1h[:, ci * CW:(ci + 1) * CW],
                                    ef[:, ci * CW:(ci + 1) * CW], op=OP.mult)
            ur = fin.tile([K, BL], f32, tag="ur")
            umr = bass.AP(tensor=um.tensor, offset=um[:].offset,
                          ap=[um[:].ap[0], [1, BL], [BL, CW // BL]])
            nc.vector.tensor_reduce(ur[:], umr, axis=mybir.AxisListType.X, op=OP.add)
            nc.vector.tensor_tensor(Uacc[:], Uacc[:], ur[:], op=OP.add)

        # emit chunk completion wall-step: chunk ci covers t in [ci*32,(ci+1)*32)
        unary_sched = {}
        for ci in range(NCH):
            t0c, t1c = ci * (T // NCH), (ci + 1) * (T // NCH) - 1
            done = max(t1c, T - 1 - t0c) + 1   # +1: emit adds land one step late
            unary_sched.setdefault(min(done, T), []).append(ci)

        # ---------------- phase 1: interleaved fwd/bwd LSTM ----------------
        # prefetch first windows
        win_cur = {"f": full_window(0, "f"), "b": full_window(15, "b")}
        win_nxt = {}
        xt_nxt = {}

        warm = ps_s.tile([1, 1], f32, tag="pssm")
        nc.tensor.matmul(warm[:], lhsT=ident[:, 0:1], rhs=ident[:, 0:1], start=True, stop=True)

        def do_pse(s):
            """Emission matmuls + delayed joint emit add for step s."""
            tf, tb = s, T - 1 - s
            tlo, thi = (tf, tb) if tf < tb else (tb, tf)
            jf = 0 if tf < tb else 1
            pse = ps_s.tile([K, 2, BL], f32, tag="pssm", name="pse")
            # single accumulation group: one bank-clear for all four matmuls
            nc.tensor.matmul(pse[:, jf], lhsT=s_wout[:, 0, :], rhs=st_h[:, 0, 0, :],
                             start=True, stop=False)
            nc.tensor.matmul(pse[:, jf], lhsT=s_wout[:, 1, :], rhs=st_h[:, 0, 1, :],
                             start=False, stop=False)
            nc.tensor.matmul(pse[:, 1 - jf], lhsT=s_wout[:, 2, :], rhs=st_h[:, 1, 0, :],
                             start=False, stop=False)
            nc.tensor.matmul(pse[:, 1 - jf], lhsT=s_wout[:, 3, :], rhs=st_h[:, 1, 1, :],
                             start=False, stop=True)
            eap = bass.AP(tensor=emit.tensor, offset=emit[:, tlo, :].offset,
                          ap=[emit[:].ap[0], [(thi - tlo) * BL, 2], [1, BL]])
            nc.vector.tensor_tensor(eap, eap, pse[:], op=OP.add)

        def burst(d, di, s):
            """Recurrence + window-injection matmuls for one stream."""
            toff = s % WIN
            tof = toff if d == "f" else WIN - 1 - toff
            psg = ps_g.tile([128, 8, BL], f32, tag=f"psg{d}", name=f"psg{d}")
            for m in range(8):
                for k in range(2):
                    nc.tensor.matmul(psg[:, m], lhsT=s_whh[d][:, k, m * 128:(m + 1) * 128],
                                     rhs=st_h[:, di, k, :], start=(m == 0 and k == 0), stop=False)
            wslice = bass.AP(tensor=win_cur[d].tensor,
                             offset=win_cur[d][:, :, tof, :].offset,
                             ap=[win_cur[d][:].ap[0], [BL * WIN, 8], [1, BL]])
            nc.tensor.matmul(psg[:], lhsT=ident[:], rhs=wslice, start=False, stop=True)
            return psg

        for s in range(T):
            blk, toff = divmod(s, WIN)

            # ---- stream f: matmuls, activations, cell update (all in place) ----
            psgf = burst("f", 0, s)
            gf = tmp.tile([128, 8, BL], f32, tag="gatf", name="gatf")
            nc.scalar.activation(gf[:], psgf[:], AF.Sigmoid)
            nc.gpsimd.tensor_scalar(gf[:, 6:8, :], gf[:, 6:8, :], 2.0, -1.0,
                                    op0=OP.mult, op1=OP.add)
            igf = tmp.tile([128, 2, BL], f32, tag="igf", name="igf")
            nc.gpsimd.tensor_tensor(igf[:], gf[:, 0:2, :], gf[:, 6:8, :], op=OP.mult)
            nc.vector.tensor_tensor(st_c[:, 0], gf[:, 2:4, :], st_c[:, 0], op=OP.mult)
            nc.vector.tensor_tensor(st_c[:, 0], st_c[:, 0], igf[:], op=OP.add)

            # ---- stream b: matmuls + emissions of step s-1 on PE ----
            psgb = burst("b", 1, s)
            if s > 0:
                do_pse(s - 1)
            gb = tmp.tile([128, 8, BL], f32, tag="gatb", name="gatb")
            nc.scalar.activation(gb[:], psgb[:], AF.Sigmoid)

            # ---- stream f tail: tanh(c), h ----
            thf = tmp.tile([128, 2, BL], f32, tag="thf", name="thf")
            nc.scalar.activation(thf[:], st_c[:, 0], AF.Tanh)
            nc.gpsimd.tensor_scalar(gb[:, 6:8, :], gb[:, 6:8, :], 2.0, -1.0,
                                    op0=OP.mult, op1=OP.add)
            igb = tmp.tile([128, 2, BL], f32, tag="igb", name="igb")
            nc.gpsimd.tensor_tensor(igb[:], gb[:, 0:2, :], gb[:, 6:8, :], op=OP.mult)
            nc.vector.tensor_tensor(st_c[:, 1], gb[:, 2:4, :], st_c[:, 1], op=OP.mult)
            nc.vector.tensor_tensor(st_h[:, 0], gf[:, 4:6, :], thf[:], op=OP.mult)
            nc.vector.tensor_tensor(st_c[:, 1], st_c[:, 1], igb[:], op=OP.add)
            thb = tmp.tile([128, 2, BL], f32, tag="thb", name="thb")
            nc.scalar.activation(thb[:], st_c[:, 1], AF.Tanh)
            nc.vector.tensor_tensor(st_h[:, 1], gb[:, 4:6, :], thb[:], op=OP.mult)

            # window prefetch/projection for the next block, one piece per step
            if blk < NW - 1:
                wf, wb = blk + 1, NW - 2 - blk
                if toff == 1:
                    xt_nxt["f"] = xtp.tile([128, 2, 512], bf16, tag="xTf", name="xTf")
                    xt_nxt["b"] = xtp.tile([128, 2, 512], bf16, tag="xTb", name="xTb")
                    win_nxt["f"] = winp.tile([128, 8, WIN, BL], bf16, tag="winf", name="winf")
                    win_nxt["b"] = winp.tile([128, 8, WIN, BL], bf16, tag="winb", name="winb")
                if toff in (2, 4, 6, 8):
                    gather_piece(xt_nxt["f"], wf, (toff - 2) // 2, "f")
                if toff in (3, 5, 7, 9):
                    gather_piece(xt_nxt["b"], wb, (toff - 3) // 2, "b")
                if 12 <= toff < 20:
                    proj_chunk(win_nxt["f"], xt_nxt["f"], "f", toff - 12)
                if 20 <= toff < 28:
                    proj_chunk(win_nxt["b"], xt_nxt["b"], "b", toff - 20)
                if toff == 28:
                    clamp_window(win_nxt["b"], wb)
                if toff == WIN - 1:
                    win_cur = dict(win_nxt)

            # interleave gold-score chunks
            if s % 32 == 16 and s // 32 < NCH // 2:
                ci = s // 32
                trans_chunk(2 * ci)
                trans_chunk(2 * ci + 1)
            for ci in unary_sched.get(s, []):
                unary_chunk(ci)

        do_pse(T - 1)
        for ci in unary_sched.get(T, []):
            unary_chunk(ci)

        # ---------------- phase 1.5: bulk exp ----------------
        ef = emit[:].rearrange("k t b -> k (t b)")
        xf = expE[:].rearrange("k t b -> k (t b)")
        for q in range(4):
            sl = slice(q * T * BL // 4, (q + 1) * T * BL // 4)
            nc.scalar.activation(xf[:, sl], ef[:, sl], AF.Exp)

        # ---------------- phase 2: CRF alpha/beta meeting at TME ----------------
        # Alpha runs in Q-space (Q_t = expA^T @ D_t) so both streams share the
        # "multiply by expE, then matmul" shape:
        #   alpha (dir 0): S0 <- masked_{m[ta]}  (expA^T @ (S0 * expE[ta])),  ta = 0..TME-1
        #   beta  (dir 1): S1 <- masked_{m[te]}  (expA   @ (S1 * expE[te])),  te = 511..TME+1
        # final: logZ = ln sum_i (expE[TME] * Q_{TME-1} * B_TME)[i] + ln-comp terms
        nc.vector.memset(S[:], 1.0)

        nrs = 0
        for kk in range(TME):
            ta = kk                # alpha uses expE[ta], mask[ta]
            te = T - kk            # beta uses expE[te], mask[te]; skip kk=0 (te=512)
            bp = tmp.tile([K, 2, BL], bf16, tag="bp", name="bp")
            psj = ps_s.tile([K, 2, BL], f32, tag="pssm", name="psj")
            if kk == 0:
                nc.vector.tensor_tensor(bp[:, 0], S[:, 0], expE[:, ta, :], op=OP.mult)
                nc.tensor.matmul(psj[:, 0], lhsT=s_eA[:], rhs=bp[:, 0], start=True, stop=True)
                nc.vector.copy_predicated(S[:, 0], mask_one(ta, K, 1), psj[:, 0])
                continue
            # joint: dir0 alpha reads expE[ta], dir1 beta reads expE[te]
            xap = bass.AP(tensor=expE.tensor, offset=expE[:, ta, :].offset,
                          ap=[expE[:].ap[0], [(te - ta) * BL, 2], [1, BL]])
            nc.vector.tensor_tensor(bp[:], S[:], xap, op=OP.mult)
            nc.tensor.matmul(psj[:, 0], lhsT=s_eA[:], rhs=bp[:, 0], start=True, stop=True)
            nc.tensor.matmul(psj[:, 1], lhsT=s_eAT[:], rhs=bp[:, 1], start=True, stop=True)
            nc.vector.copy_predicated(S[:], mask_joint(ta, te, K, 1), psj[:])

            if kk % RESC == 0 and kk >= 8 and kk <= 248:
                # rescale both streams; ln terms batched at the end
                pss = ps_s.tile([1, 2 * BL], f32, tag="pssm", name="pss")
                nc.tensor.matmul(pss[:], lhsT=ones[0:K, 0:1],
                                 rhs=S[:].rearrange("k d b -> k (d b)"), start=True, stop=True)
                nc.vector.tensor_copy(lnbuf[:, nrs].rearrange("o d b -> o (d b)"), pss[:])
                rr = tmp.tile([1, 2 * BL], f32, tag="rr")
                nc.vector.reciprocal(rr[:], pss[:])
                psr = ps_s.tile([K, 2 * BL], f32, tag="pssm", name="psr")
                nc.tensor.matmul(psr[:], lhsT=ones[0:1, 0:K], rhs=rr[:], start=True, stop=True)
                sc = tmp.tile([K, 2, BL], f32, tag="sc")
                nc.vector.tensor_tensor(sc[:].rearrange("k d b -> k (d b)"),
                                        S[:].rearrange("k d b -> k (d b)"), psr[:], op=OP.mult)
                nc.vector.copy_predicated(S[:], mask_joint(ta, te, K, 1), sc[:])
                nrs += 1
        assert nrs == NRESC, nrs

        # final compensated rescale (bounds the final Ln input); plain scaling,
        # the lnmask row gates S1's compensation to columns that use it
        pss = ps_s.tile([1, 2 * BL], f32, tag="pssm", name="pssf")
        nc.tensor.matmul(pss[:], lhsT=ones[0:K, 0:1],
                         rhs=S[:].rearrange("k d b -> k (d b)"), start=True, stop=True)
        nc.vector.tensor_copy(lnbuf[:, NRESC].rearrange("o d b -> o (d b)"), pss[:])
        rr = tmp.tile([1, 2 * BL], f32, tag="rr")
        nc.vector.reciprocal(rr[:], pss[:])
        psr = ps_s.tile([K, 2 * BL], f32, tag="pssm", name="psrf")
        nc.tensor.matmul(psr[:], lhsT=ones[0:1, 0:K], rhs=rr[:], start=True, stop=True)
        nc.vector.tensor_tensor(S[:].rearrange("k d b -> k (d b)"),
                                S[:].rearrange("k d b -> k (d b)"), psr[:], op=OP.mult)

        # ---------------- finalize ----------------
        # logZ = ln(sum_i D[i]*B[i]) + sum(masked ln rescale terms)
        lns = fin.tile([1, (NRESC + 1) * 2 * BL], f32, tag="lns")
        nc.scalar.activation(lns[:], lnbuf[:].rearrange("o r d b -> o (r d b)"), AF.Ln)
        nc.vector.tensor_tensor(lns[:], lns[:], s_lnm[:], op=OP.mult)
        lnred = fin.tile([1, BL], f32, tag="lnred")
        lnsr = bass.AP(tensor=lns.tensor, offset=lns[:].offset,
                       ap=[lns[:].ap[0], [1, BL], [BL, (NRESC + 1) * 2]])
        nc.vector.tensor_reduce(lnred[:], lnsr, axis=mybir.AxisListType.X, op=OP.add)

        # combine vector V: active cols (len > TME) use expE[TME]*beta,
        # frozen cols (len <= TME) use w* (recovers sum_i D_i from Q-space)
        psR = ps_s.tile([K, BL], f32, tag="pssm", name="psR")
        nc.tensor.matmul(psR[:], lhsT=s_ws[:], rhs=ones[0:1, 0:BL], start=True, stop=True)
        V = fin.tile([K, BL], f32, tag="V")
        nc.vector.tensor_copy(V[:], psR[:])
        EV = fin.tile([K, BL], f32, tag="EV")
        nc.vector.tensor_tensor(EV[:], expE[:, TME, :], S[:, 1], op=OP.mult)
        nc.vector.copy_predicated(V[:], mask_one(TME, K, 1), EV[:])
        zt = fin.tile([K, BL], f32, tag="zt")
        nc.vector.tensor_tensor(zt[:], S[:, 0], V[:], op=OP.mult)
        psz = ps_s.tile([1, BL], f32, tag="pssm", name="psz")
        nc.tensor.matmul(psz[:], lhsT=ones[0:K, 0:1], rhs=zt[:], start=True, stop=True)
        logZ = fin.tile([1, BL], f32, tag="logZ")
        nc.scalar.activation(logZ[:], psz[:], AF.Ln)
        nc.vector.tensor_tensor(logZ[:], logZ[:], lnred[:], op=OP.add)

        # gold score = colsum(Uacc) + colsum(TRacc)
        nc.vector.tensor_tensor(Uacc[:], Uacc[:], TRacc[:], op=OP.add)
        psu = ps_s.tile([1, BL], f32, tag="pssm", name="psu")
        nc.tensor.matmul(psu[:], lhsT=ones[0:K, 0:1], rhs=Uacc[:], start=True, stop=True)

        res = fin.tile([1, BL], f32, tag="res")
        nc.vector.tensor_tensor(res[:], logZ[:], psu[:], op=OP.subtract)
        nc.sync.dma_start(out=out_loss[:], in_=res[:])

    nc.compile()
    return nc, names


def _prep_core(inputs, kcore):
    """Per-core host-side input prep (index plumbing + layout shuffles)."""
    import ml_dtypes
    bf = ml_dtypes.bfloat16
    s = slice(kcore * BL, (kcore + 1) * BL)
    sent = np.asarray(inputs["sentences"][s])          # (16, 512) i32
    tags = np.asarray(inputs["tags"][s])               # (16, 512) i32
    mask = (sent != PAD_IDX)                           # (16, 512)

    # gate-row permutation [i,f,g,o] -> [i,f,o,g]; g rows scaled x2 so that
    # tanh(g) can be computed as 2*sigmoid(2g)-1 with a single Sigmoid op
    perm = np.concatenate([np.arange(0, 2 * H), np.arange(3 * H, 4 * H),
                           np.arange(2 * H, 3 * H)])
    gsc = np.ones((4 * H, 1), np.float32)
    gsc[3 * H:] = 2.0

    toks = sent.reshape(BL, NW, WIN).transpose(1, 2, 0).reshape(T * BL, 1)
    oh = (tags[:, :, None] == np.arange(K)[None, None, :])
    t1h = (oh & mask[:, :, None]).transpose(2, 1, 0).reshape(K, T * BL)
    tnxm = np.zeros((BL, T, K), np.float32)
    tnxm[:, :-1, :] = (oh[:, 1:, :] & mask[:, 1:, None]).astype(np.float32)
    tnx = tnxm.transpose(2, 1, 0).reshape(K, T * BL)

    # lnmask: mask value at the rescale checkpoints, layout (r, dir, b)
    lnm = np.zeros((NRESC + 1, 2, BL), np.float32)
    for r in range(NRESC):
        kk = 8 * (r + 1)
        lnm[r, 0] = mask[:, kk].astype(np.float32)       # alpha gate: mask[ta=kk]
        lnm[r, 1] = mask[:, T - kk].astype(np.float32)   # beta gate: mask[te=512-kk]
    lnm[NRESC, 0] = 1.0                                  # final rescale: S0 always
    lnm[NRESC, 1] = mask[:, TME].astype(np.float32)      # S1 only if used

    A = np.asarray(inputs["transition"], np.float32)

    def wperm(w):
        return np.ascontiguousarray((np.asarray(w)[perm] * gsc).T).astype(bf)

    def bperm(b):
        return np.asarray(b)[perm] * gsc[:, 0]

    m = {
        "toks": toks.astype(np.int32),
        "masku": mask.T.astype(np.uint8).reshape(1, T * BL),
        "invmu": (~mask).T.astype(np.uint8).reshape(1, T * BL),
        "t1h": t1h.astype(bf),
        "tnx": tnx.astype(bf),
        "lnmask": lnm.reshape(1, (NRESC + 1) * 2 * BL),
        "emb": np.asarray(inputs["embedding"]).astype(bf),
        "wih_f": wperm(inputs["w_ih_f"]),
        "wih_b": wperm(inputs["w_ih_b"]),
        "whh_f": wperm(inputs["w_hh_f"]),
        "whh_b": wperm(inputs["w_hh_b"]),
        "bih_f": np.ascontiguousarray(bperm(inputs["b_f"]).reshape(8, 128).T).astype(np.float32),
        "bih_b": np.ascontiguousarray(bperm(inputs["b_b"]).reshape(8, 128).T).astype(np.float32),
        "bihT_f": bperm(inputs["b_f"]).reshape(1, 8 * 128).astype(bf),
        "bihT_b": bperm(inputs["b_b"]).reshape(1, 8 * 128).astype(bf),
        "woutT": np.ascontiguousarray(np.asarray(inputs["w_out"]).T.reshape(4, 128, K)).astype(bf),
        "bout": np.asarray(inputs["b_out"]).reshape(K, 1).astype(np.float32),
        "expA": np.exp(A).astype(bf),
        "expAT": np.ascontiguousarray(np.exp(A).T).astype(bf),
        "Abf": A.astype(bf),
        "wstar": np.linalg.solve(np.exp(A), np.ones(K)).reshape(1, K).astype(np.float32),
    }
    return m


def kernel(**inputs):
    from concourse.bass_utils import run_bass_kernel_spmd

    if "prog" not in _cache:
        _cache["prog"] = _build_program()
    nc, names = _cache["prog"]

    in_maps = []
    for kcore in range(NCORES):
        m = _prep_core(inputs, kcore)
        in_maps.append({names[kk]: vv for kk, vv in m.items()})

    res = run_bass_kernel_spmd(nc, in_maps, core_ids=list(range(NCORES)),
                               **_cache.get("run_kwargs", {}))
    out = np.concatenate([r[names["out"]].reshape(BL) for r in res.results])
    _cache["last_results"] = res
    return out.astype(np.float32)


# revision 60
# speedup vs baseline: 1.2072x; 1.2072x over previous
"""BiLSTM-CRF loss kernel for Trainium2 (8 NeuronCores, data-parallel over batch).

v2 design (per core, BL=16 sequences):
  Phase 1 (512 wall-steps): fwd LSTM (t=s) and bwd LSTM (t=511-s) run as two
    interleaved streams so every engine pipelines across streams.
    - Gate order host-permuted to [i,f,o,g] so one Sigmoid op covers i,f,o.
    - Input projection xw is windowed (32 steps) and injected into the gate
      PSUM accumulation via an identity matmul (PE is cheap, DVE is not).
    - Emissions pre-biased once; both streams add their w_out matmul via a
      single joint DVE op per step.
    - Only Sigmoid/Tanh on ACT -> zero activation-table thrash.
    - Gold-score (unary+transition) reductions interleaved into phase-1 slack.
  Phase 2 (~256 wall-steps): CRF log-partition via two interleaved exp-space
    recursions on bulk-exp'd emissions: alpha upward (t=1..256) and beta
    downward (t=510..256), meeting at t*=256. Periodic rescale; the log
    compensation terms are stored and Ln'd once at the end.
"""

import numpy as np

PAD_IDX = 0
VOCAB, K, E, H = 30000, 20, 256, 256
B, T = 128, 512
NCORES = 8
BL = B // NCORES          # 16 sequences per core
WIN = 32                  # proj window (time steps)
NW = T // WIN             # 16 windows
RESC = 8                  # CRF rescale interval (wall steps)
TME = 256                 # CRF meeting point: logZ = ln(sum D_TME * B_TME)
NRESC = 31                # rescales at k=8,16,...,248

_cache = {}


def _build_program():
    from contextlib import ExitStack
    import concourse.bass as bass
    import concourse.bacc as bacc
    import concourse.tile as tile
    from concourse import mybir
    from concourse.masks import make_identity

    f32 = mybir.dt.float32
    bf16 = mybir.dt.bfloat16
    i32 = mybir.dt.int32
    u8 = mybir.dt.uint8
    AF = mybir.ActivationFunctionType
    OP = mybir.AluOpType

    nc = bacc.Bacc(None, target_bir_lowering=False, debug=False)
    names = {}

    with ExitStack() as ctx:
        tc = ctx.enter_context(tile.TileContext(nc))
        dram = ctx.enter_context(tc.tile_pool(name="dram", bufs=1, space="DRAM"))

        def din(key, shape, dt=f32):
            t = dram.tile(shape, dt, kind="ExternalInput", name=key)
            names[key] = t.tensor.name
            return t

        emb = din("emb", [VOCAB, E], bf16)
        toks = din("toks", [T * BL, 1], i32)            # window-major token ids
        masku = din("masku", [1, T * BL], u8)           # col = t*16+b
        invmu = din("invmu", [1, T * BL], u8)           # 1 - mask
        t1h = din("t1h", [K, T * BL], bf16)             # one-hot(tag)*mask
        tnx = din("tnx", [K, T * BL], bf16)             # shifted one-hot*mask
        lnmask = din("lnmask", [1, (NRESC + 1) * 2 * BL])  # mask at rescale times
        wih = {d: din(f"wih_{d}", [E, 4 * H], bf16) for d in "fb"}
        whh = {d: din(f"whh_{d}", [E, 4 * H], bf16) for d in "fb"}
        bih = {d: din(f"bih_{d}", [128, 8]) for d in "fb"}
        bihT = {d: din(f"bihT_{d}", [1, 8 * 128], bf16) for d in "fb"}
        woutT = din("woutT", [4, 128, K], bf16)         # chunks: Fk0,Fk1,Bk0,Bk1
        bout = din("bout", [K, 1])
        expA = din("expA", [K, K], bf16)                # exp(transition)
        expAT = din("expAT", [K, K], bf16)              # exp(transition).T
        Abf = din("Abf", [K, K], bf16)                  # transition (bf16)
        wstar = din("wstar", [1, K])                    # 1^T (expA^T)^{-1}
        out_loss = dram.tile([1, BL], f32, kind="ExternalOutput")
        names["out"] = out_loss.tensor.name

        sg = ctx.enter_context(tc.tile_pool(name="sg", bufs=1))       # singles
        tmp = ctx.enter_context(tc.tile_pool(name="tmp", bufs=4))     # step temps
        gat = ctx.enter_context(tc.tile_pool(name="gat", bufs=4))     # gather tiles
        fin = ctx.enter_context(tc.tile_pool(name="fin", bufs=3))     # finalize
        ps_g = ctx.enter_context(tc.tile_pool(name="ps_g", bufs=2, space="PSUM"))
        ps_w = ctx.enter_context(tc.tile_pool(name="ps_w", bufs=1, space="PSUM"))
        ps_t = ctx.enter_context(tc.tile_pool(name="ps_t", bufs=1, space="PSUM"))
        ps_s = ctx.enter_context(tc.tile_pool(name="ps_s", bufs=2, space="PSUM"))

        # ---- resident SBUF tensors ----
        s_wih = {d: sg.tile([128, 2, 4 * H], bf16, tag=f"wih{d}", name=f"wih{d}") for d in "fb"}
        s_whh = {d: sg.tile([128, 2, 4 * H], bf16, tag=f"whh{d}", name=f"whh{d}") for d in "fb"}
        for d in "fb":
            nc.sync.dma_start(out=s_wih[d][:], in_=wih[d][:].rearrange("(k p) m -> p k m", p=128))
            nc.sync.dma_start(out=s_whh[d][:], in_=whh[d][:].rearrange("(k p) m -> p k m", p=128))
        s_bih = {d: sg.tile([128, 8], f32, tag=f"bih{d}", name=f"bih{d}") for d in "fb"}
        s_bihT = {d: sg.tile([1, 8, 128], bf16, tag=f"bihT{d}", name=f"bihT{d}") for d in "fb"}
        for d in "fb":
            nc.sync.dma_start(out=s_bih[d][:], in_=bih[d][:])
            nc.sync.dma_start(out=s_bihT[d][:].rearrange("o m p -> o (m p)"), in_=bihT[d][:])
        onesb = sg.tile([1, 512], bf16, tag="onesb")
        nc.vector.memset(onesb[:], 1.0)
        s_wout = sg.tile([128, 4, K], bf16, tag="wout")
        nc.sync.dma_start(out=s_wout[:], in_=woutT[:].rearrange("c p k -> p c k"))
        s_bout = sg.tile([K, 1], f32, tag="bout")
        nc.sync.dma_start(out=s_bout[:], in_=bout[:])
        s_eA = sg.tile([K, K], bf16, tag="eA")
        nc.sync.dma_start(out=s_eA[:], in_=expA[:])
        s_eAT = sg.tile([K, K], bf16, tag="eAT")
        nc.sync.dma_start(out=s_eAT[:], in_=expAT[:])
        s_A = sg.tile([K, K], bf16, tag="A")
        nc.sync.dma_start(out=s_A[:], in_=Abf[:])
        s_ws = sg.tile([1, K], f32, tag="ws")
        nc.sync.dma_start(out=s_ws[:], in_=wstar[:])
        s_t1h = sg.tile([K, T * BL], bf16, tag="t1h")
        nc.sync.dma_start(out=s_t1h[:], in_=t1h[:])
        s_tnx = sg.tile([K, T * BL], bf16, tag="tnx")
        nc.sync.dma_start(out=s_tnx[:], in_=tnx[:])
        s_lnm = sg.tile([1, (NRESC + 1) * 2 * BL], f32, tag="lnm")
        nc.sync.dma_start(out=s_lnm[:], in_=lnmask[:])

        ones = sg.tile([128, K], f32, tag="ones")
        nc.vector.memset(ones[:], 1.0)
        ident = sg.tile([128, 128], bf16, tag="ident")
        make_identity(nc, ident[:])

        # mask replica: (128, T, BL), col = t*16+b, broadcast across partitions
        maskrep = sg.tile([128, T, BL], u8, tag="maskrep")
        nc.sync.dma_start(
            out=maskrep[:],
            in_=bass.AP(tensor=masku.tensor, offset=masku[:].offset,
                        ap=[[0, 128], [BL, T], [1, BL]]),
        )
        invrep = sg.tile([128, T, BL], u8, tag="invrep")
        nc.sync.dma_start(
            out=invrep[:],
            in_=bass.AP(tensor=invmu.tensor, offset=invmu[:].offset,
                        ap=[[0, 128], [BL, T], [1, BL]]),
        )
        neg50 = sg.tile([128, 1], bf16, tag="neg50")
        nc.vector.memset(neg50[:], -50.0)

        # token indices for gathers (128 per column)
        NT128 = T * BL // 128
        idxall = sg.tile([128, NT128], i32, tag="idxall")
        nc.sync.dma_start(out=idxall[:],
                          in_=bass.AP(tensor=toks.tensor, offset=toks[:].offset,
                                      ap=[[1, 128], [128, NT128]]))

        # emissions (f32) and their exp; pre-bias emit with b_out
        emit = sg.tile([K, T, BL], f32, tag="emit")
        emf0 = emit[:].rearrange("k t b -> k (t b)")
        nc.vector.memset(emf0, 0.0)
        nc.vector.tensor_scalar_add(emf0, emf0, s_bout[:, 0:1])
        expE = emit  # exp taken in place after phase 1 (unary reads are done by then)

        # per-direction rotating transposed-input windows and projection windows
        xtp = ctx.enter_context(tc.tile_pool(name="xtp", bufs=2))
        winp = ctx.enter_context(tc.tile_pool(name="winp", bufs=2))

        # LSTM states, layout [128, dir(2), k(2), BL]
        st_h = sg.tile([128, 2, 2, BL], bf16, tag="st_h")
        st_c = sg.tile([128, 2, 2, BL], f32, tag="st_c")
        nc.vector.memset(st_h[:], 0.0)
        nc.vector.memset(st_c[:], 0.0)

        # CRF state S[:, 0]=alpha D, S[:, 1]=beta B; ln-compensation buffer
        S = sg.tile([K, 2, BL], f32, tag="S")
        lnbuf = sg.tile([1, NRESC + 1, 2, BL], f32, tag="lnbuf")
        Uacc = sg.tile([K, BL], f32, tag="Uacc")
        nc.vector.memset(Uacc[:], 0.0)
        TRacc = sg.tile([K, BL], f32, tag="TRacc")
        nc.vector.memset(TRacc[:], 0.0)

        def gather_piece(xT, w, g, d):
            """Gather+transpose one 128-token piece of window w."""
            j = w * 4 + g
            xg = gat.tile([128, E], bf16, tag="xg")
            nc.gpsimd.indirect_dma_start(
                out=xg[:], out_offset=None, in_=emb[:],
                in_offset=bass.IndirectOffsetOnAxis(ap=idxall[:, j:j + 1], axis=0),
            )
            for k in range(2):
                pst = ps_t.tile([128, 128], bf16, tag="pst")
                nc.tensor.transpose(out=pst[:], in_=xg[:, k * 128:(k + 1) * 128],
                                    identity=ident[:])
                if (g + k) % 2 == 0:
                    nc.vector.tensor_copy(xT[:, k, g * 128:(g + 1) * 128], pst[:])
                else:
                    nc.scalar.activation(xT[:, k, g * 128:(g + 1) * 128], pst[:], AF.Copy)

        def proj_chunk(win, xT, d, m):
            """One m-chunk of the window projection (+bias via matmul)."""
            psw = ps_w.tile([128, 512], f32, tag="psw")
            for k in range(2):
                nc.tensor.matmul(psw[:], lhsT=s_wih[d][:, k, m * 128:(m + 1) * 128],
                                 rhs=xT[:, k, :], start=(k == 0), stop=False)
            nc.tensor.matmul(psw[:], lhsT=s_bihT[d][:, m, :], rhs=onesb[:],
                             start=False, stop=True)
            wv = win[:, m].rearrange("p t b -> p (t b)")
            if m % 2 == 0:
                nc.scalar.activation(wv, psw[:], AF.Copy)
            else:
                nc.vector.tensor_copy(wv, psw[:])

        def clamp_window(win, w):
            """Clamp i-gate pre-activation to -50 at padded positions (bwd)."""
            pred = bass.AP(tensor=invrep.tensor,
                           offset=invrep[0:128, w * WIN, 0:BL].offset,
                           ap=[invrep[:].ap[0], [0, 2], [BL, WIN], [1, BL]])
            data = bass.AP(tensor=neg50.tensor, offset=neg50[:].offset,
                           ap=[neg50[:].ap[0], [0, 2], [0, WIN], [0, BL]])
            nc.vector.copy_predicated(win[:, 0:2], pred, data)

        def full_window(w, d):
            xT = xtp.tile([128, 2, 512], bf16, tag=f"xT{d}", name=f"xT{d}")
            for g in range(4):
                gather_piece(xT, w, g, d)
            win = winp.tile([128, 8, WIN, BL], bf16, tag=f"win{d}", name=f"win{d}")
            for m in range(8):
                proj_chunk(win, xT, d, m)
            if d == "b":
                clamp_window(win, w)
            return win

        # mask AP helpers --------------------------------------------------
        def mask_joint(tlo, thi, parts, reps):
            """(parts, 2, reps, BL) AP over maskrep: dir0 at t=tlo, dir1 at t=thi."""
            base = maskrep[0:parts, tlo, 0:BL]
            return bass.AP(tensor=base.tensor, offset=base.offset,
                           ap=[base.ap[0], [(thi - tlo) * BL, 2], [0, reps], [1, BL]])

        def mask_one(t, parts, reps):
            base = maskrep[0:parts, t, 0:BL]
            if reps == 1:
                return base
            return bass.AP(tensor=base.tensor, offset=base.offset,
                           ap=[base.ap[0], [0, reps], [1, BL]])

        # -------- gold-score chunk work (interleaved into phase 1) --------
        NCH = 16
        CW = T * BL // NCH          # 512 cols per chunk

        def trans_chunk(ci):
            """TRacc += reduce_t(A[tag, :] * shifted-onehot) for chunk ci."""
            psa = ps_w.tile([K, CW], f32, tag="psw", name="psa")
            nc.tensor.matmul(psa[:], lhsT=s_A[:], rhs=s_t1h[:, ci * CW:(ci + 1) * CW],
                             start=True, stop=True)
            um = fin.tile([K, CW], f32, tag="um")
            nc.vector.tensor_tensor(um[:], psa[:], s_tnx[:, ci * CW:(ci + 1) * CW], op=OP.mult)
            ur = fin.tile([K, BL], f32, tag="ur")
            umr = bass.AP(tensor=um.tensor, offset=um[:].offset,
                          ap=[um[:].ap[0], [1, BL], [BL, CW // BL]])
            nc.vector.tensor_reduce(ur[:], umr, axis=mybir.AxisListType.X, op=OP.add)
            nc.vector.tensor_tensor(TRacc[:], TRacc[:], ur[:], op=OP.add)

        def unary_chunk(ci):
            """Uacc += reduce_t(onehot * emit) for chunk ci (emit must be complete)."""
            um = fin.tile([K, CW], f32, tag="um")
            ef = emit[:].rearrange("k t b -> k (t b)")
            nc.gpsimd.tensor_tensor(um[:], s_t1h[:, ci * CW:(ci + 1) * CW],
                                    ef[:, ci * CW:(ci + 1) * CW], op=OP.mult)
            ur = fin.tile([K, BL], f32, tag="ur")
            umr = bass.AP(tensor=um.tensor, offset=um[:].offset,
                          ap=[um[:].ap[0], [1, BL], [BL, CW // BL]])
            nc.vector.tensor_reduce(ur[:], umr, axis=mybir.AxisListType.X, op=OP.add)
            nc.vector.tensor_tensor(Uacc[:], Uacc[:], ur[:], op=OP.add)

        # emit chunk completion wall-step: chunk ci covers t in [ci*32,(ci+1)*32)
        unary_sched = {}
        for ci in range(NCH):
            t0c, t1c = ci * (T // NCH), (ci + 1) * (T // NCH) - 1
            done = max(t1c, T - 1 - t0c) + 1   # +1: emit adds land one step late
            unary_sched.setdefault(min(done, T), []).append(ci)

        # ---------------- phase 1: interleaved fwd/bwd LSTM ----------------
        # prefetch first windows
        win_cur = {"f": full_window(0, "f"), "b": full_window(15, "b")}
        win_nxt = {}
        xt_nxt = {}

        warm = ps_s.tile([1, 1], f32, tag="pssm")
        nc.tensor.matmul(warm[:], lhsT=ident[:, 0:1], rhs=ident[:, 0:1], start=True, stop=True)

        def do_pse(s):
            """Emission matmuls + delayed joint emit add for step s."""
            tf, tb = s, T - 1 - s
            tlo, thi = (tf, tb) if tf < tb else (tb, tf)
            jf = 0 if tf < tb else 1
            pse = ps_s.tile([K, 2, BL], f32, tag="pssm", name="pse")
            # single accumulation group: one bank-clear for all four matmuls
            nc.tensor.matmul(pse[:, jf], lhsT=s_wout[:, 0, :], rhs=st_h[:, 0, 0, :],
                             start=True, stop=False)
            nc.tensor.matmul(pse[:, jf], lhsT=s_wout[:, 1, :], rhs=st_h[:, 0, 1, :],
                             start=False, stop=False)
            nc.tensor.matmul(pse[:, 1 - jf], lhsT=s_wout[:, 2, :], rhs=st_h[:, 1, 0, :],
                             start=False, stop=False)
            nc.tensor.matmul(pse[:, 1 - jf], lhsT=s_wout[:, 3, :], rhs=st_h[:, 1, 1, :],
                             start=False, stop=True)
            eap = bass.AP(tensor=emit.tensor, offset=emit[:, tlo, :].offset,
                          ap=[emit[:].ap[0], [(thi - tlo) * BL, 2], [1, BL]])
            nc.vector.tensor_tensor(eap, eap, pse[:], op=OP.add)

        def burst(d, di, s):
            """Recurrence + window-injection matmuls for one stream."""
            toff = s % WIN
            tof = toff if d == "f" else WIN - 1 - toff
            psg = ps_g.tile([128, 8, BL], f32, tag=f"psg{d}", name=f"psg{d}")
            # two accumulation groups: i,f,g chunks first so their sigmoid (and
            # the g-path) starts while the o-chunk matmuls still stream
            def wslice(m0, mn):
                return bass.AP(tensor=win_cur[d].tensor,
                               offset=win_cur[d][:, m0:, tof, :].offset,
                               ap=[win_cur[d][:].ap[0], [BL * WIN, mn], [1, BL]])
            for m in range(6):
                for k in range(2):
                    nc.tensor.matmul(psg[:, m], lhsT=s_whh[d][:, k, m * 128:(m + 1) * 128],
                                     rhs=st_h[:, di, k, :], start=(m == 0 and k == 0), stop=False)
            nc.tensor.matmul(psg[:, 0:6], lhsT=ident[:], rhs=wslice(0, 6),
                             start=False, stop=True)
            for m in range(6, 8):
                for k in range(2):
                    nc.tensor.matmul(psg[:, m], lhsT=s_whh[d][:, k, m * 128:(m + 1) * 128],
                                     rhs=st_h[:, di, k, :], start=(m == 6 and k == 0), stop=False)
            nc.tensor.matmul(psg[:, 6:8], lhsT=ident[:], rhs=wslice(6, 2),
                             start=False, stop=True)
            return psg

        for s in range(T):
            blk, toff = divmod(s, WIN)

            # ---- stream f: matmuls, activations, cell update (all in place) ----
            psgf = burst("f", 0, s)
            gf = tmp.tile([128, 8, BL], f32, tag="gatf", name="gatf")
            nc.scalar.activation(gf[:, 0:6, :], psgf[:, 0:6, :], AF.Sigmoid)
            nc.gpsimd.tensor_scalar(gf[:, 4:6, :], gf[:, 4:6, :], 2.0, -1.0,
                                    op0=OP.mult, op1=OP.add)
            igf = tmp.tile([128, 2, BL], f32, tag="igf", name="igf")
            nc.gpsimd.tensor_tensor(igf[:], gf[:, 0:2, :], gf[:, 4:6, :], op=OP.mult)
            nc.scalar.activation(gf[:, 6:8, :], psgf[:, 6:8, :], AF.Sigmoid)
            nc.vector.tensor_tensor(st_c[:, 0], gf[:, 2:4, :], st_c[:, 0], op=OP.mult)
            nc.vector.tensor_tensor(st_c[:, 0], st_c[:, 0], igf[:], op=OP.add)

            # ---- stream b: matmuls + emissions of step s-1 on PE ----
            psgb = burst("b", 1, s)
            if s > 0:
                do_pse(s - 1)
            gb = tmp.tile([128, 8, BL], f32, tag="gatb", name="gatb")
            nc.scalar.activation(gb[:, 0:6, :], psgb[:, 0:6, :], AF.Sigmoid)

            # ---- stream f tail: tanh(c), h ----
            thf = tmp.tile([128, 2, BL], f32, tag="thf", name="thf")
            nc.scalar.activation(thf[:], st_c[:, 0], AF.Tanh)
            nc.gpsimd.tensor_scalar(gb[:, 4:6, :], gb[:, 4:6, :], 2.0, -1.0,
                                    op0=OP.mult, op1=OP.add)
            igb = tmp.tile([128, 2, BL], f32, tag="igb", name="igb")
            nc.gpsimd.tensor_tensor(igb[:], gb[:, 0:2, :], gb[:, 4:6, :], op=OP.mult)
            nc.scalar.activation(gb[:, 6:8, :], psgb[:, 6:8, :], AF.Sigmoid)
            nc.vector.tensor_tensor(st_c[:, 1], gb[:, 2:4, :], st_c[:, 1], op=OP.mult)
            nc.vector.tensor_tensor(st_h[:, 0], gf[:, 6:8, :], thf[:], op=OP.mult)
            nc.vector.tensor_tensor(st_c[:, 1], st_c[:, 1], igb[:], op=OP.add)
            thb = tmp.tile([128, 2, BL], f32, tag="thb", name="thb")
            nc.scalar.activation(thb[:], st_c[:, 1], AF.Tanh)
            nc.vector.tensor_tensor(st_h[:, 1], gb[:, 6:8, :], thb[:], op=OP.mult)

            # window prefetch/projection for the next block, one piece per step
            if blk < NW - 1:
                wf, wb = blk + 1, NW - 2 - blk
                if toff == 1:
                    xt_nxt["f"] = xtp.tile([128, 2, 512], bf16, tag="xTf", name="xTf")
                    xt_nxt["b"] = xtp.tile([128, 2, 512], bf16, tag="xTb", name="xTb")
                    win_nxt["f"] = winp.tile([128, 8, WIN, BL], bf16, tag="winf", name="winf")
                    win_nxt["b"] = winp.tile([128, 8, WIN, BL], bf16, tag="winb", name="winb")
                if toff in (2, 4, 6, 8):
                    gather_piece(xt_nxt["f"], wf, (toff - 2) // 2, "f")
                if toff in (3, 5, 7, 9):
                    gather_piece(xt_nxt["b"], wb, (toff - 3) // 2, "b")
                if 12 <= toff < 20:
                    proj_chunk(win_nxt["f"], xt_nxt["f"], "f", toff - 12)
                if 20 <= toff < 28:
                    proj_chunk(win_nxt["b"], xt_nxt["b"], "b", toff - 20)
                if toff == 28:
                    clamp_window(win_nxt["b"], wb)
                if toff == WIN - 1:
                    win_cur = dict(win_nxt)

            # interleave gold-score chunks
            if s % 32 == 16 and s // 32 < NCH // 2:
                ci = s // 32
                trans_chunk(2 * ci)
                trans_chunk(2 * ci + 1)
            for ci in unary_sched.get(s, []):
                unary_chunk(ci)

        do_pse(T - 1)
        for ci in unary_sched.get(T, []):
            unary_chunk(ci)

        # ---------------- phase 1.5: bulk exp ----------------
        ef = emit[:].rearrange("k t b -> k (t b)")
        xf = expE[:].rearrange("k t b -> k (t b)")
        for q in range(4):
            sl = slice(q * T * BL // 4, (q + 1) * T * BL // 4)
            nc.scalar.activation(xf[:, sl], ef[:, sl], AF.Exp)

        # ---------------- phase 2: CRF alpha/beta meeting at TME ----------------
        # Alpha runs in Q-space (Q_t = expA^T @ D_t) so both streams share the
        # "multiply by expE, then matmul" shape:
        #   alpha (dir 0): S0 <- masked_{m[ta]}  (expA^T @ (S0 * expE[ta])),  ta = 0..TME-1
        #   beta  (dir 1): S1 <- masked_{m[te]}  (expA   @ (S1 * expE[te])),  te = 511..TME+1
        # final: logZ = ln sum_i (expE[TME] * Q_{TME-1} * B_TME)[i] + ln-comp terms
        nc.vector.memset(S[:], 1.0)

        nrs = 0
        for kk in range(TME):
            ta = kk                # alpha uses expE[ta], mask[ta]
            te = T - kk            # beta uses expE[te], mask[te]; skip kk=0 (te=512)
            bp = tmp.tile([K, 2, BL], bf16, tag="bp", name="bp")
            psj = ps_s.tile([K, 2, BL], f32, tag="pssm", name="psj")
            if kk == 0:
                nc.vector.tensor_tensor(bp[:, 0], S[:, 0], expE[:, ta, :], op=OP.mult)
                nc.tensor.matmul(psj[:, 0], lhsT=s_eA[:], rhs=bp[:, 0], start=True, stop=True)
                nc.vector.copy_predicated(S[:, 0], mask_one(ta, K, 1), psj[:, 0])
                continue
            # joint: dir0 alpha reads expE[ta], dir1 beta reads expE[te]
            xap = bass.AP(tensor=expE.tensor, offset=expE[:, ta, :].offset,
                          ap=[expE[:].ap[0], [(te - ta) * BL, 2], [1, BL]])
            nc.vector.tensor_tensor(bp[:], S[:], xap, op=OP.mult)
            nc.tensor.matmul(psj[:, 0], lhsT=s_eA[:], rhs=bp[:, 0], start=True, stop=True)
            nc.tensor.matmul(psj[:, 1], lhsT=s_eAT[:], rhs=bp[:, 1], start=True, stop=True)
            nc.vector.copy_predicated(S[:], mask_joint(ta, te, K, 1), psj[:])

            if kk % RESC == 0 and kk >= 8 and kk <= 248:
                # rescale both streams; ln terms batched at the end
                pss = ps_s.tile([1, 2 * BL], f32, tag="pssm", name="pss")
                nc.tensor.matmul(pss[:], lhsT=ones[0:K, 0:1],
                                 rhs=S[:].rearrange("k d b -> k (d b)"), start=True, stop=True)
                nc.vector.tensor_copy(lnbuf[:, nrs].rearrange("o d b -> o (d b)"), pss[:])
                rr = tmp.tile([1, 2 * BL], f32, tag="rr")
                nc.vector.reciprocal(rr[:], pss[:])
                psr = ps_s.tile([K, 2 * BL], f32, tag="pssm", name="psr")
                nc.tensor.matmul(psr[:], lhsT=ones[0:1, 0:K], rhs=rr[:], start=True, stop=True)
                sc = tmp.tile([K, 2, BL], f32, tag="sc")
                nc.vector.tensor_tensor(sc[:].rearrange("k d b -> k (d b)"),
                                        S[:].rearrange("k d b -> k (d b)"), psr[:], op=OP.mult)
                nc.vector.copy_predicated(S[:], mask_joint(ta, te, K, 1), sc[:])
                nrs += 1
        assert nrs == NRESC, nrs

        # final compensated rescale (bounds the final Ln input); plain scaling,
        # the lnmask row gates S1's compensation to columns that use it
        pss = ps_s.tile([1, 2 * BL], f32, tag="pssm", name="pssf")
        nc.tensor.matmul(pss[:], lhsT=ones[0:K, 0:1],
                         rhs=S[:].rearrange("k d b -> k (d b)"), start=True, stop=True)
        nc.vector.tensor_copy(lnbuf[:, NRESC].rearrange("o d b -> o (d b)"), pss[:])
        rr = tmp.tile([1, 2 * BL], f32, tag="rr")
        nc.vector.reciprocal(rr[:], pss[:])
        psr = ps_s.tile([K, 2 * BL], f32, tag="pssm", name="psrf")
        nc.tensor.matmul(psr[:], lhsT=ones[0:1, 0:K], rhs=rr[:], start=True, stop=True)
        nc.vector.tensor_tensor(S[:].rearrange("k d b -> k (d b)"),
                                S[:].rearrange("k d b -> k (d b)"), psr[:], op=OP.mult)

        # ---------------- finalize ----------------
        # logZ = ln(sum_i D[i]*B[i]) + sum(masked ln rescale terms)
        lns = fin.tile([1, (NRESC + 1) * 2 * BL], f32, tag="lns")
        nc.scalar.activation(lns[:], lnbuf[:].rearrange("o r d b -> o (r d b)"), AF.Ln)
        nc.vector.tensor_tensor(lns[:], lns[:], s_lnm[:], op=OP.mult)
        lnred = fin.tile([1, BL], f32, tag="lnred")
        lnsr = bass.AP(tensor=lns.tensor, offset=lns[:].offset,
                       ap=[lns[:].ap[0], [1, BL], [BL, (NRESC + 1) * 2]])
        nc.vector.tensor_reduce(lnred[:], lnsr, axis=mybir.AxisListType.X, op=OP.add)

        # combine vector V: active cols (len > TME) use expE[TME]*beta,
        # frozen cols (len <= TME) use w* (recovers sum_i D_i from Q-space)
        psR = ps_s.tile([K, BL], f32, tag="pssm", name="psR")
        nc.tensor.matmul(psR[:], lhsT=s_ws[:], rhs=ones[0:1, 0:BL], start=True, stop=True)
        V = fin.tile([K, BL], f32, tag="V")
        nc.vector.tensor_copy(V[:], psR[:])
        EV = fin.tile([K, BL], f32, tag="EV")
        nc.vector.tensor_tensor(EV[:], expE[:, TME, :], S[:, 1], op=OP.mult)
        nc.vector.copy_predicated(V[:], mask_one(TME, K, 1), EV[:])
        zt = fin.tile([K, BL], f32, tag="zt")
        nc.vector.tensor_tensor(zt[:], S[:, 0], V[:], op=OP.mult)
        psz = ps_s.tile([1, BL], f32, tag="pssm", name="psz")
        nc.tensor.matmul(psz[:], lhsT=ones[0:K, 0:1], rhs=zt[:], start=True, stop=True)
        logZ = fin.tile([1, BL], f32, tag="logZ")
        nc.scalar.activation(logZ[:], psz[:], AF.Ln)
        nc.vector.tensor_tensor(logZ[:], logZ[:], lnred[:], op=OP.add)

        # gold score = colsum(Uacc) + colsum(TRacc)
        nc.vector.tensor_tensor(Uacc[:], Uacc[:], TRacc[:], op=OP.add)
        psu = ps_s.tile([1, BL], f32, tag="pssm", name="psu")
        nc.tensor.matmul(psu[:], lhsT=ones[0:K, 0:1], rhs=Uacc[:], start=True, stop=True)

        res = fin.tile([1, BL], f32, tag="res")
        nc.vector.tensor_tensor(res[:], logZ[:], psu[:], op=OP.subtract)
        nc.sync.dma_start(out=out_loss[:], in_=res[:])

    nc.compile()
    return nc, names


def _prep_core(inputs, kcore):
    """Per-core host-side input prep (index plumbing + layout shuffles)."""
    import ml_dtypes
    bf = ml_dtypes.bfloat16
    s = slice(kcore * BL, (kcore + 1) * BL)
    sent = np.asarray(inputs["sentences"][s])          # (16, 512) i32
    tags = np.asarray(inputs["tags"][s])               # (16, 512) i32
    mask = (sent != PAD_IDX)                           # (16, 512)

    # native gate order [i,f,g,o]; g rows scaled x2 so that tanh(g) can be
    # computed as 2*sigmoid(2g)-1 with Sigmoid ops only
    perm = np.arange(4 * H)
    gsc = np.ones((4 * H, 1), np.float32)
    gsc[2 * H:3 * H] = 2.0

    toks = sent.reshape(BL, NW, WIN).transpose(1, 2, 0).reshape(T * BL, 1)
    oh = (tags[:, :, None] == np.arange(K)[None, None, :])
    t1h = (oh & mask[:, :, None]).transpose(2, 1, 0).reshape(K, T * BL)
    tnxm = np.zeros((BL, T, K), np.float32)
    tnxm[:, :-1, :] = (oh[:, 1:, :] & mask[:, 1:, None]).astype(np.float32)
    tnx = tnxm.transpose(2, 1, 0).reshape(K, T * BL)

    # lnmask: mask value at the rescale checkpoints, layout (r, dir, b)
    lnm = np.zeros((NRESC + 1, 2, BL), np.float32)
    for r in range(NRESC):
        kk = 8 * (r + 1)
        lnm[r, 0] = mask[:, kk].astype(np.float32)       # alpha gate: mask[ta=kk]
        lnm[r, 1] = mask[:, T - kk].astype(np.float32)   # beta gate: mask[te=512-kk]
    lnm[NRESC, 0] = 1.0                                  # final rescale: S0 always
    lnm[NRESC, 1] = mask[:, TME].astype(np.float32)      # S1 only if used

    A = np.asarray(inputs["transition"], np.float32)

    def wperm(w):
        return np.ascontiguousarray((np.asarray(w)[perm] * gsc).T).astype(bf)

    def bperm(b):
        return np.asarray(b)[perm] * gsc[:, 0]

    m = {
        "toks": toks.astype(np.int32),
        "masku": mask.T.astype(np.uint8).reshape(1, T * BL),
        "invmu": (~mask).T.astype(np.uint8).reshape(1, T * BL),
        "t1h": t1h.astype(bf),
        "tnx": tnx.astype(bf),
        "lnmask": lnm.reshape(1, (NRESC + 1) * 2 * BL),
        "emb": np.asarray(inputs["embedding"]).astype(bf),
        "wih_f": wperm(inputs["w_ih_f"]),
        "wih_b": wperm(inputs["w_ih_b"]),
        "whh_f": wperm(inputs["w_hh_f"]),
        "whh_b": wperm(inputs["w_hh_b"]),
        "bih_f": np.ascontiguousarray(bperm(inputs["b_f"]).reshape(8, 128).T).astype(np.float32),
        "bih_b": np.ascontiguousarray(bperm(inputs["b_b"]).reshape(8, 128).T).astype(np.float32),
        "bihT_f": bperm(inputs["b_f"]).reshape(1, 8 * 128).astype(bf),
        "bihT_b": bperm(inputs["b_b"]).reshape(1, 8 * 128).astype(bf),
        "woutT": np.ascontiguousarray(np.asarray(inputs["w_out"]).T.reshape(4, 128, K)).astype(bf),
        "bout": np.asarray(inputs["b_out"]).reshape(K, 1).astype(np.float32),
        "expA": np.exp(A).astype(bf),
        "expAT": np.ascontiguousarray(np.exp(A).T).astype(bf),
        "Abf": A.astype(bf),
        "wstar": np.linalg.solve(np.exp(A), np.ones(K)).reshape(1, K).astype(np.float32),
    }
    return m


def kernel(**inputs):
    from concourse.bass_utils import run_bass_kernel_spmd

    if "prog" not in _cache:
        _cache["prog"] = _build_program()
    nc, names = _cache["prog"]

    in_maps = []
    for kcore in range(NCORES):
        m = _prep_core(inputs, kcore)
        in_maps.append({names[kk]: vv for kk, vv in m.items()})

    res = run_bass_kernel_spmd(nc, in_maps, core_ids=list(range(NCORES)),
                               **_cache.get("run_kwargs", {}))
    out = np.concatenate([r[names["out"]].reshape(BL) for r in res.results])
    _cache["last_results"] = res
    return out.astype(np.float32)
